# revision 25
# baseline (speedup 1.0000x reference)
"""Trainium2 Bass kernel for nn_ContextAttention_30270929502708.

Reference computation (N=2, C=64, M=4800, K=64):
  cat   = [scene_rgb (bc over m); query (bc over k)]         (N,2C,M,K)
  h     = relu(bn0(W0 @ cat))
  cat2  = [h; scene_xyz*mask (bc over m)]                    (N,C+3,M,K)
  h1    = relu(bn1a(W1a @ cat2)); h1 = relu(bn1b(W1b @ h1))
  feat  = h1 + Wskip @ cat2
  out   = Wout @ [max_k feat; pre_xyz]                       (N,3,M)

Restructure: every conv contribution from k-only tensors (scene_rgb,
scene_xyz*mask) is a tiny (C,K) constant, precomputed on host with BN
scales folded into the weights:
  h  = relu(A0h[c,k] + B0[c,m]),  B0 = W0q' @ query   (on device)
  h1 = relu(W1a' @ h + D1h[c,k]); h2 = relu(W1b' @ h1 + t1b[c])
  feat_mk = h2 + Wskip_c @ h + Dkh[c,k]
Partition layout packs both batches: p = n*64 + c (block-diag weights).
Per-core m-shard of 600; loop over k with per-k [P,1] scalars; the
max over k is a running scalar_tensor_tensor accumulate on DVE; h2 is
injected into the skip PSUM bank via an identity matmul on PE.
"""

import functools

import numpy as np

N, C, M, K = 2, 64, 4800, 64
EPS = 1e-5
NCORES = 8
MCORE = M // NCORES  # 600
MH = MCORE // 2      # 300 (matmul half, fits one PSUM bank)

TRACE = False
last_exec_time_ns = None


def _blkdiag(a, b):
    out = np.zeros((a.shape[0] + b.shape[0], a.shape[1] + b.shape[1]), np.float32)
    out[: a.shape[0], : a.shape[1]] = a
    out[a.shape[0] :, a.shape[1] :] = b
    return out


def _fold(g, b, m, v):
    s = g / np.sqrt(v + EPS)
    return s.astype(np.float32), (b - m * s).astype(np.float32)


@functools.lru_cache(maxsize=16)
def _build_program(bench_reps=1, variant="full"):
    import contextlib

    import concourse.mybir as mybir
    import concourse.tile as tile
    from concourse import bacc

    fp32 = mybir.dt.float32
    AT = mybir.ActivationFunctionType
    OP = mybir.AluOpType

    nc = bacc.Bacc("TRN2", target_bir_lowering=False, debug=False,
                   num_devices=NCORES)

    bf16 = mybir.dt.bfloat16
    BF16_NAMES = ("w1aT", "w1bT", "wskipT", "ident")
    din = {}
    for name, shape in [
        ("query_s", [128, MCORE]), ("prexyz_s", [6, MCORE]),
        ("a0h", [128, K]), ("d1h", [128, K]), ("dkh", [128, K]),
        ("t1b", [128, 1]),
        ("wq0T", [128, 128]), ("w1aT", [128, 128]), ("w1bT", [128, 128]),
        ("wskipT", [128, 128]), ("ident", [128, 128]),
        ("woutT", [128, 6]), ("woutxT", [6, 6]),
    ]:
        dt = bf16 if name in BF16_NAMES else fp32
        din[name] = nc.dram_tensor(name, shape, dt, kind="ExternalInput")
    out_s = nc.dram_tensor("out_s", [6, MCORE], fp32, kind="ExternalOutput")

    with tile.TileContext(nc) as tc:
        with (
            tc.tile_pool(name="const", bufs=1) as cp,
            tc.tile_pool(name="hp", bufs=3) as hp,
            tc.tile_pool(name="h1p", bufs=2) as h1p,
            tc.tile_pool(name="h2p", bufs=3) as h2p,
            tc.tile_pool(name="pp1", bufs=1, space="PSUM") as pp1,
            tc.tile_pool(name="pp2", bufs=1, space="PSUM") as pp2,
            tc.tile_pool(name="pp3", bufs=2, space="PSUM") as pp3,
        ):
            # ---- load constants / per-core inputs into SBUF ----
            sb = {}
            for name, shape in [
                ("query_s", [128, 2, MH]), ("prexyz_s", [6, 2, MH]),
                ("a0h", [128, K]), ("d1h", [128, K]), ("dkh", [128, K]),
                ("t1b", [128, 1]),
                ("wq0T", [128, 128]), ("w1aT", [128, 128]),
                ("w1bT", [128, 128]), ("wskipT", [128, 128]),
                ("ident", [128, 128]), ("woutT", [128, 6]),
                ("woutxT", [6, 6]),
            ]:
                t = cp.tile(shape, bf16 if name in BF16_NAMES else fp32,
                            tag=name)
                src = din[name][:, :]
                if len(shape) == 3:
                    src = src.rearrange("p (a b) -> p a b", a=2)
                nc.sync.dma_start(out=t, in_=src)
                sb[name] = t

            # ---- B0 = Wq0' @ query  (PSUM -> SBUF) ----
            pb = pp1.tile([128, 2, 512], fp32, tag="p1")
            for i in range(2):
                nc.tensor.matmul(out=pb[:, i, :MH], lhsT=sb["wq0T"],
                                 rhs=sb["query_s"][:, i, :], start=True, stop=True)
            b0 = cp.tile([128, 2, MH], fp32, tag="b0")
            nc.scalar.copy(out=b0, in_=pb[:, :, :MH])

            # ---- feat ping-pong accumulators ----
            feats = [cp.tile([128, 2, MH], fp32, tag=f"feat{i}", name=f"feat{i}")
                     for i in range(2)]
            nc.vector.memset(feats[0], -1e30)

            # ---- main loop over k ----
            # bench_reps>1 unrolls extra sweeps: the feat max-accumulate is
            # idempotent, so repeats are numerically safe and expose the
            # loop's HW time as a slope vs reps.
            use_pe = variant in ("full", "pe_only", "no_inject")
            use_inject = variant in ("full",)
            use_dve = variant in ("full", "no_pe", "dve_only", "no_inject")
            use_act = variant in ("full", "no_pe", "act_only", "no_inject")

            # inject[k] + feat-max[k] are deferred by one k (software
            # pipelining) so PE never stalls waiting for ACT's h2[k].
            pending = []

            def flush_pending():
                p3_p, h2_p, k_p = pending.pop()
                for i in range(2):
                    nc.tensor.matmul(out=p3_p[:, i, :MH], lhsT=sb["ident"],
                                     rhs=h2_p[:, i, :], start=False, stop=True)
                s, d = feats[k_p % 2], feats[(k_p + 1) % 2]
                nc.vector.scalar_tensor_tensor(
                    out=d, in0=p3_p[:, :, :MH],
                    scalar=sb["dkh"][:, k_p : k_p + 1],
                    in1=s, op0=OP.add, op1=OP.max)

            for _rep in range(bench_reps):
                for k in range(K):
                    h = (hp.tile([128, 2, MH], bf16, tag="h", name="h")
                         if use_dve else None)
                    if use_dve:
                        # h = relu(B0 + a0h[:,k])
                        nc.vector.tensor_scalar(
                            out=h, in0=b0, scalar1=sb["a0h"][:, k : k + 1],
                            scalar2=0.0, op0=OP.add, op1=OP.max)
                    rhs1 = h if use_dve else b0

                    p1 = (pp1.tile([128, 2, 512], fp32, tag="p1", name="p1")
                          if use_pe or use_act else None)
                    if use_pe:
                        for i in range(2):
                            nc.tensor.matmul(out=p1[:, i, :MH], lhsT=sb["w1aT"],
                                             rhs=rhs1[:, i, :], start=True,
                                             stop=True)
                    h1 = (h1p.tile([128, 2, MH], bf16, tag="h1", name="h1")
                          if use_act else None)
                    if use_act:
                        nc.scalar.activation(out=h1, in_=p1[:, :, :MH],
                                             func=AT.Relu,
                                             bias=sb["d1h"][:, k : k + 1],
                                             scale=1.0)
                    rhs2 = h1 if use_act else b0

                    p2 = (pp2.tile([128, 2, 512], fp32, tag="p2", name="p2")
                          if use_pe or use_act else None)
                    if use_pe:
                        for i in range(2):
                            nc.tensor.matmul(out=p2[:, i, :MH], lhsT=sb["w1bT"],
                                             rhs=rhs2[:, i, :], start=True,
                                             stop=True)
                    h2 = (h2p.tile([128, 2, MH], bf16, tag="h2", name="h2")
                          if use_act else None)
                    if use_act:
                        nc.scalar.activation(out=h2, in_=p2[:, :, :MH],
                                             func=AT.Relu,
                                             bias=sb["t1b"][:, 0:1], scale=1.0)
                    rhs3 = h2 if use_act else b0

                    p3 = (pp3.tile([128, 2, 512], fp32, tag="p3", name="p3")
                          if use_pe or use_inject or use_dve else None)
                    if use_pe:
                        for i in range(2):
                            nc.tensor.matmul(out=p3[:, i, :MH],
                                             lhsT=sb["wskipT"],
                                             rhs=rhs1[:, i, :], start=True,
                                             stop=not use_inject)
                    if use_inject:
                        if pending:
                            flush_pending()
                        pending.append((p3, rhs3, k))
                    elif use_dve:
                        # feat = max(feat, p3 + dkh[:,k])
                        src, dst = feats[k % 2], feats[(k + 1) % 2]
                        nc.vector.scalar_tensor_tensor(
                            out=dst, in0=p3[:, :, :MH],
                            scalar=sb["dkh"][:, k : k + 1],
                            in1=src, op0=OP.add, op1=OP.max)
                if pending:
                    flush_pending()

            feat = feats[K % 2]
            # ---- out conv: Wout_c @ feat + Wout_x @ pre_xyz ----
            po = pp2.tile([6, 2, 512], fp32, tag="p2", name="po")
            for i in range(2):
                nc.tensor.matmul(out=po[:, i, :MH], lhsT=sb["woutT"],
                                 rhs=feat[:, i, :], start=True, stop=False)
                nc.tensor.matmul(out=po[:, i, :MH], lhsT=sb["woutxT"],
                                 rhs=sb["prexyz_s"][:, i, :], start=False, stop=True)
            out_sb = cp.tile([6, 2, MH], fp32, tag="out_sb")
            nc.scalar.copy(out=out_sb, in_=po[:, :, :MH])
            nc.sync.dma_start(out=out_s[:, :].rearrange("p (a b) -> p a b", a=2),
                              in_=out_sb)

    nc.compile()
    return nc


def _host_prep(query_rgb_feat, scene_rgb_feat, scene_xyz, pre_xyz, mask,
               W0, g0, b0, m0, v0, W1a, g1a, b1a, m1a, v1a,
               W1b, g1b, b1b, m1b, v1b, Wskip, Wout):
    f32 = np.float32
    s0, t0 = _fold(g0, b0, m0, v0)
    s1a, t1a = _fold(g1a, b1a, m1a, v1a)
    s1b, t1b = _fold(g1b, b1b, m1b, v1b)

    scene = np.asarray(scene_rgb_feat, f32)[:, :, 0, :]          # (N,C,K)
    sxm = (np.asarray(scene_xyz, f32) * np.asarray(mask, f32))[:, :, 0, :]  # (N,3,K)
    query = np.asarray(query_rgb_feat, f32)[:, :, :, 0]          # (N,C,M)

    W0 = np.asarray(W0, f32)
    W1a = np.asarray(W1a, f32)
    W1b = np.asarray(W1b, f32)
    Wskip = np.asarray(Wskip, f32)
    Wout = np.asarray(Wout, f32)

    # per-batch (C,K) constants with BN folded
    a0h = np.concatenate(
        [s0[:, None] * (W0[:, :C] @ scene[n]) + t0[:, None] for n in range(N)], 0)
    d1h = np.concatenate(
        [s1a[:, None] * (W1a[:, C:] @ sxm[n]) + t1a[:, None] for n in range(N)], 0)
    dkh = np.concatenate([Wskip[:, C:] @ sxm[n] for n in range(N)], 0)

    w0q = s0[:, None] * W0[:, C:]
    w1a_c = s1a[:, None] * W1a[:, :C]
    w1b_c = s1b[:, None] * W1b

    import ml_dtypes
    bf16 = ml_dtypes.bfloat16
    consts = {
        "a0h": np.ascontiguousarray(a0h, f32),
        "d1h": np.ascontiguousarray(d1h, f32),
        "dkh": np.ascontiguousarray(dkh, f32),
        "t1b": np.ascontiguousarray(np.tile(t1b, 2)[:, None], f32),
        "wq0T": _blkdiag(w0q.T, w0q.T),
        "w1aT": _blkdiag(w1a_c.T, w1a_c.T).astype(bf16),
        "w1bT": _blkdiag(w1b_c.T, w1b_c.T).astype(bf16),
        "wskipT": _blkdiag(Wskip[:, :C].T, Wskip[:, :C].T).astype(bf16),
        "ident": np.eye(128, dtype=f32).astype(bf16),
        "woutT": _blkdiag(Wout[:, :C].T, Wout[:, :C].T),
        "woutxT": _blkdiag(Wout[:, C:].T, Wout[:, C:].T),
    }
    query_p = query.reshape(N * C, M)                       # (128, M)
    prexyz_p = np.asarray(pre_xyz, f32).reshape(N * 3, M)   # (6, M)
    return consts, query_p, prexyz_p


def _run_via_pjrt(nc, in_maps, bench_iters=0, _return_fn=False):
    """Execute the Bass module on NCORES cores via PJRT (axon-friendly).

    Mirrors bass2jax.run_bass_via_pjrt's multi-core path but keeps the
    jitted callable so repeated timed executions are possible.
    Returns (per_core_results, per_iter_seconds_list).
    """
    import time

    import jax
    import jax.numpy as jnp
    from jax.sharding import Mesh, NamedSharding, PartitionSpec
    from jax.experimental.shard_map import shard_map

    import concourse.mybir as mybir
    from concourse import bass2jax

    bass2jax.install_neuronx_cc_hook()
    assert nc.dbg_addr is None
    partition_name = (nc.partition_id_tensor.name
                      if nc.partition_id_tensor else None)

    in_names, out_names, out_avals, zero_outs = [], [], [], []
    for alloc in nc.m.functions[0].allocations:
        if not isinstance(alloc, mybir.MemoryLocationSet):
            continue
        name = alloc.memorylocations[0].name
        if alloc.kind == "ExternalInput":
            if name != partition_name:
                in_names.append(name)
        elif alloc.kind == "ExternalOutput":
            shape = tuple(alloc.tensor_shape)
            dtype = mybir.dt.np(alloc.dtype)
            out_names.append(name)
            out_avals.append(jax.core.ShapedArray(shape, dtype))
            zero_outs.append(np.zeros(shape, dtype))
    n_params = len(in_names)
    n_outs = len(out_avals)
    all_in_names = in_names + out_names
    if partition_name is not None:
        all_in_names.append(partition_name)
    donate = tuple(range(n_params, n_params + n_outs))

    def _body(*args):
        operands = list(args)
        if partition_name is not None:
            operands.append(bass2jax.partition_id_tensor())
        outs = bass2jax._bass_exec_p.bind(
            *operands,
            out_avals=tuple(out_avals),
            in_names=tuple(all_in_names),
            out_names=tuple(out_names),
            lowering_input_output_aliases=(),
            sim_require_finite=True,
            sim_require_nnan=True,
            nc=nc,
        )
        return tuple(outs)

    devices = jax.devices()[:NCORES]
    mesh = Mesh(np.asarray(devices), ("core",))
    spec = PartitionSpec("core")
    in_specs = (spec,) * (n_params + n_outs)
    out_specs = (spec,) * n_outs
    sharded = jax.jit(
        shard_map(_body, mesh=mesh, in_specs=in_specs, out_specs=out_specs,
                  check_rep=False),
        donate_argnums=donate, keep_unused=True)

    concat_in = [
        np.concatenate([np.asarray(in_maps[c][name]) for c in range(NCORES)], 0)
        for name in in_names
    ]
    concat_zero_shapes = [(NCORES * z.shape[0], *z.shape[1:]) for z in zero_outs]
    sh = NamedSharding(mesh, spec)
    dev_in = [jax.device_put(a, sh) for a in concat_in]

    def _zeros():
        zs = [jax.device_put(np.zeros(s, np.float32), sh)
              for s in concat_zero_shapes]
        jax.block_until_ready(zs)
        return zs

    out_arrs = sharded(*dev_in, *_zeros())
    jax.block_until_ready(out_arrs)

    def timed_once():
        zs = _zeros()
        t0 = time.perf_counter()
        r = sharded(*dev_in, *zs)
        jax.block_until_ready(r)
        return time.perf_counter() - t0

    times = [timed_once() for _ in range(bench_iters)]

    results = [
        {name: np.asarray(out_arrs[i]).reshape(NCORES, *out_avals[i].shape)[c]
         for i, name in enumerate(out_names)}
        for c in range(NCORES)
    ]
    if _return_fn:
        return results, times, timed_once
    return results, times


def _make_in_maps(inputs):
    consts, query_p, prexyz_p = _host_prep(**inputs)
    in_maps = []
    for c in range(NCORES):
        sl = slice(c * MCORE, (c + 1) * MCORE)
        im = dict(consts)
        im["query_s"] = np.ascontiguousarray(query_p[:, sl])
        im["prexyz_s"] = np.ascontiguousarray(prexyz_p[:, sl])
        in_maps.append(im)
    return in_maps


def _assemble(results):
    out = np.empty((N, 3, M), np.float32)
    for c in range(NCORES):
        sl = slice(c * MCORE, (c + 1) * MCORE)
        out[:, :, sl] = results[c]["out_s"].reshape(N, 3, MCORE)
    return out


def kernel(**inputs):
    nc = _build_program(1)
    in_maps = _make_in_maps(inputs)
    results, _ = _run_via_pjrt(nc, in_maps, bench_iters=0)
    return _assemble(results)


def _make_runner(nc, in_maps):
    """Build the jitted sharded callable once; return (run_once, results_fn)."""
    import time

    import jax

    state = {}

    def run_once():
        if "fn" not in state:
            results, times, fn = _run_via_pjrt(nc, in_maps, bench_iters=1,
                                               _return_fn=True)
            state["fn"] = fn
            state["results"] = results
            return times[0]
        return state["fn"]()

    return run_once, lambda: state["results"]


def bench_loop_ns(inputs, r_lo=1, r_hi=33, iters=30, verbose=False,
                  variant="full"):
    """Estimate the HW time of one 64-k main-loop sweep via the slope of
    wall time vs in-kernel unrolled repetition count. Paired back-to-back
    (lo, hi) runs with a median over the differences cancel axon RPC
    drift and outliers."""
    in_maps = _make_in_maps(inputs)
    run_lo, _ = _make_runner(_build_program(r_lo, variant), in_maps)
    run_hi, _ = _make_runner(_build_program(r_hi, variant), in_maps)
    diffs = []
    for i in range(iters):
        if i % 2 == 0:
            a, b = run_lo(), run_hi()
            diffs.append(b - a)
        else:
            b, a = run_hi(), run_lo()
            diffs.append(b - a)
    diffs_ns = np.array(diffs) / (r_hi - r_lo) * 1e9
    if verbose:
        print("per-pair slope estimates (ns):",
              np.percentile(diffs_ns, [10, 25, 50, 75, 90]).astype(int))
    return float(np.median(diffs_ns))
